# revision 30
# baseline (speedup 1.0000x reference)
"""Trainium2 Bass kernel for multi-head attention (B=4, N=2048, C=256, H=16).

Sharding: 8 cores, core = 2*b + g handles batch b and heads 8g..8g+7 (two
groups g2 of 4 heads).  Each core computes its heads' attention plus a
partial output projection; the host sums the two partials per batch and
adds b_proj (plus the folded v-bias term, see below).

Bottleneck analysis: softmax exp over 8 heads x 2048^2 = 33.5M elements
per core is elementwise-engine bound (1 elem/cycle/lane reading fp32 from
PSUM).  The kernel therefore SPLITS the exp work between ScalarE (true
exp activation) and VectorE (Schraudolph bit-trick exp: bf16 bits =
int16(round(A*s + B)), A = 128*log2(e), B = 128*127).  The bit-trick's
~4% per-element error largely cancels in the softmax normalization
(denominators use the same approximate values); measured end-to-end
rel_fro ~1.1e-2 at a ~39% VectorE share (gate is 2e-2).

Per-core layout (channels on partitions, "transposed"):
  xT  [128, cc, n]  host-pretransposed, DMA'd outside the timed loop
  qT/kT fp16 spread: head lj of a group occupies partitions
      32lj..32lj+16, = W^T @ xT (fp32r self-loading matmuls, copied out
      as fp16);  q gets +bq, k bias is DROPPED (exactly cancels in
      softmax), v bias is folded into the host-side output bias
      (sum(p)=Z normalization makes it additive).
  v natural [tok, vd] via matmul(lhsT=xT-chunk, rhs=Wv-chunk) -> vaug
      bf16 [keys, kt, 8, 17] with ones in column 16 (row-sum trick).
  scores S^T[key, q] per (g2, nn, kt, pr): 2 row-group fp16 matmuls
      (K=16, tile_position=(32lj, 0)) into one [128, 1024] PSUM tile.
  exp -> bf16 P tile: ScalarE activation OR VectorE tensor_scalar into
      an int16 bitcast of the bf16 tile (engine chosen per tile by a
      Bresenham split with N_ACT/256 on ScalarE).
  attnv: col-group bf16 matmuls (tile_position=(0, 32lj)) accumulate
      all 4 heads into ONE PSUM bank per (g2, nn); a K=1 zero-row
      "zero_fill" matmul clears/claims the bank first (start=True wipes
      a whole bank, and the Tile scheduler may reorder disjoint-region
      writers, so per-chain start flags are unsafe); the chains then
      accumulate with start=False, which is reorder-safe.
  Emission is software-pipelined (PIPE_DEPTH): scores for tile t+2 are
      emitted before attnv for tile t so the static per-engine schedule
      keeps PE busy while ScalarE/VectorE exponentiate.
  normalize: sums row broadcast via sel matmul, reciprocal_approx_fast,
      in-place multiply; projection fp32r, partial out DMA'd per token
      tile.  A post-schedule pass (_dedup_ldweights) drops LDWEIGHTS
      that reload weights already resident in the same PE row-group.
"""

import numpy as np

import concourse.bass as bass
import concourse.mybir as mybir
import concourse.tile as tile
from concourse import bacc

F32 = mybir.dt.float32
F32R = mybir.dt.float32r
FP16 = mybir.dt.float16
BF16 = mybir.dt.bfloat16
I16 = mybir.dt.int16
EXPF = mybir.ActivationFunctionType.Exp

P = 128
B, N_FULL, C, H, D = 4, 2048, 256, 16, 16
CC = C // P          # 2 channel chunks
KT = N_FULL // P     # 16 key tiles
QC = 512             # q-chunk
NQ = N_FULL // QC    # 4 q-chunks
NCORES = 8

LOG2E = 1.4426950408889634
SCH_A = float(np.float32(128.0 * LOG2E))
SCH_B = float(np.float32(128.0 * 127.0))

# Number of the 256 exp tiles handled by ScalarE (rest go to VectorE via
# the Schraudolph bit-trick).  Balances ScalarE vs VectorE busy time.
N_ACT = 156

# Ablation knobs (timing experiments only; break numerics when < full size):
# free-dim used by each component's instructions.
ABLATE = {"sc_n": QC, "exp_n": 2 * QC, "av_n": QC}

# Scores-ahead-of-attnv distance in the emitted instruction stream.
PIPE_DEPTH = 2

# q/k + scores matmul dtype: FP16 (discrete LDWEIGHTS path) or F32R
# (self-loading weight path, no separate load instruction).
SC_DT = FP16

_NC_CACHE: dict = {}
LAST_RESULT = None  # BassKernelResults of the most recent run (for test.py)
TIMING_REPS = 1  # >1 repeats the compute on-device (timing); output unchanged


def _act_assignment(n_act=N_ACT):
    """Bresenham-spread a boolean per exp-tile index: True -> ScalarE."""
    flags = []
    for idx in range(256):
        flags.append(((idx + 1) * n_act) // 256 != (idx * n_act) // 256)
    return flags


def build(n_act=N_ACT, reps=1):
    on_act = _act_assignment(n_act)

    nc = bacc.Bacc()
    xt_d = nc.dram_tensor("xt", [P, CC, N_FULL], F32R, kind="ExternalInput")
    wq_d = nc.dram_tensor("wq", [2, C, P], F32R, kind="ExternalInput")
    wk_d = nc.dram_tensor("wk", [2, C, P], F32R, kind="ExternalInput")
    wv_d = nc.dram_tensor("wv", [C, P], F32R, kind="ExternalInput")
    bq_d = nc.dram_tensor("bq", [2, P], F32, kind="ExternalInput")
    wp_d = nc.dram_tensor("wp", [2, P, C], F32R, kind="ExternalInput")
    sel_d = nc.dram_tensor("sel", [P, P], F32R, kind="ExternalInput")
    out_d = nc.dram_tensor("out", [N_FULL, C], F32, kind="ExternalOutput")

    with tile.TileContext(nc) as tc:
        with (
            tc.tile_pool(name="const", bufs=1) as const,
            tc.tile_pool(name="otp", bufs=8) as otp,
            tc.tile_pool(name="work", bufs=4) as work,
            tc.tile_pool(name="ptp", bufs=6) as ptp,
            tc.tile_pool(name="flow", bufs=3, space="PSUM") as flow,
            tc.tile_pool(name="acc", bufs=2, space="PSUM") as acc,
        ):
            # ---------------- loads (outside the timed loop) ----------------
            def staged_load(name, shape, dt, src_ap):
                sb = const.tile(shape, dt, name=f"{name}_sb")
                nc.sync.dma_start(sb[:], src_ap)
                return sb

            wq_sb = staged_load(
                "wq", [P, 2, CC, P], F32R,
                wq_d[:].rearrange("g (cc p) f -> p g cc f", p=P),
            )
            wk_sb = staged_load(
                "wk", [P, 2, CC, P], F32R,
                wk_d[:].rearrange("g (cc p) f -> p g cc f", p=P),
            )
            wv_sb = staged_load(
                "wv", [P, CC, P], F32R, wv_d[:].rearrange("(cc p) f -> p cc f", p=P)
            )
            bq_sb = staged_load("bq", [P, 2], F32, bq_d[:].rearrange("g p -> p g"))
            wp_sb = staged_load("wp", [P, 2, C], F32R, wp_d[:].rearrange("g p c -> p g c"))
            sel_sb = staged_load("sel", [P, P], F32R, sel_d[:])
            # zero row for the PSUM-clearing dummy matmuls (see _build_body)
            zero_sb = const.tile([P, QC], BF16, name="zero_sb")
            nc.vector.memset(zero_sb[:], 0.0)
            # x, pre-transposed on host; chunked DMAs to engage parallel queues
            xt_sb = const.tile([P, CC, N_FULL], F32R, name="xt_sb")
            for cc in range(CC):
                for half in range(2):
                    sl = slice(half * (N_FULL // 2), (half + 1) * (N_FULL // 2))
                    nc.sync.dma_start(xt_sb[:, cc, sl], xt_d[:, cc, sl])

            from contextlib import nullcontext

            # PE/DVE loop bodies exceed one 256-instruction IRAM block, so
            # without branch hints their back-edge I$-misses (~3-4us stall)
            # every timing iteration
            loop_ctx = (
                tc.For_i(
                    0, reps, 1,
                    hint_engines=(mybir.EngineType.PE, mybir.EngineType.DVE),
                )
                if reps > 1
                else nullcontext()
            )
            with loop_ctx:
                _build_body(
                    nc, tc, const, otp, work, ptp, flow, acc, on_act,
                    xt_sb, wq_sb, wk_sb, wv_sb, wp_sb, sel_sb, bq_sb, zero_sb,
                    out_d,
                )
    _dedup_ldweights(nc)
    nc.finalize()
    return nc


def _build_body(
    nc, tc, const, otp, work, ptp, flow, acc, on_act,
    xt_sb, wq_sb, wk_sb, wv_sb, wp_sb, sel_sb, bq_sb, zero_sb, out_d,
):
    def zero_fill(psum_ap, ncols):
        """K=1 matmul of a zero row: clears the bank's has_written bits,
        writes zeros with the bits set over the whole region, and (by
        overlapping every later accumulating matmul) forces WAW ordering.
        Accumulation chains into disjoint regions of a shared bank then use
        start=False throughout, which is reorder-safe (pure adds)."""
        nc.tensor.matmul(
            psum_ap, zero_sb[0:1, 0:psum_ap.partition_size()],
            zero_sb[0:1, 0:ncols], start=True, stop=True,
        )
    qt = [const.tile([P, N_FULL], SC_DT, name=f"qt{g}") for g in range(2)]
    kt = [const.tile([P, N_FULL], SC_DT, name=f"kt{g}") for g in range(2)]
    vaug = const.tile([P, KT, 8, 17], BF16, name="vaug")
    nc.vector.memset(vaug[:, :, :, 16], 1.0)

    def prologue_qk(g2):
        for c in range(NQ):
            sl = slice(c * QC, (c + 1) * QC)
            for w_sb, dst, bias in ((wq_sb, qt[g2], True), (wk_sb, kt[g2], False)):
                ps = flow.tile([P, QC], F32, tag="flow", name="ps")
                for cc in range(CC):
                    nc.tensor.matmul(
                        ps[:],
                        w_sb[:, g2, cc, :],
                        xt_sb[:, cc, sl],
                        start=(cc == 0),
                        stop=(cc == CC - 1),
                    )
                nc.vector.tensor_copy(dst[:, sl], ps[:])
                if bias:
                    nc.vector.tensor_scalar_add(
                        dst[:, sl], dst[:, sl], bq_sb[:, g2 : g2 + 1]
                    )

    def prologue_v():
        for t in range(KT):
            ps = flow.tile([P, P], F32, tag="flow", name="psv")
            for cc in range(CC):
                nc.tensor.matmul(
                    ps[:],
                    xt_sb[:, cc, t * P : (t + 1) * P],
                    wv_sb[:, cc, :],
                    start=(cc == 0),
                    stop=(cc == CC - 1),
                )
            nc.vector.tensor_copy(
                vaug[:, t, :, 0:16], ps[:].rearrange("p (h d) -> p h d", d=16)
            )

    ot_tiles = {}

    def attention(g2):
        # Software-pipelined emission: scores matmuls run PIPE_DEPTH tiles
        # ahead of the attnv matmuls in the static per-engine schedule, so
        # PE computes upcoming scores while ScalarE/VectorE exponentiate
        # and the exp wait is already satisfied at each attnv pair.
        from concourse.tile_rust import add_dep_helper

        pending = []  # [(pt, at, kt_i, pr), ...]
        last_pe = [None]

        def flush_one():
            if not pending:
                return
            pt, p_at, p_kt, p_pr = pending.pop(0)
            avn = ABLATE["av_n"]
            for j2 in range(2):
                lj = 2 * p_pr + j2
                last_pe[0] = nc.tensor.matmul(
                    p_at[32 * lj : 32 * lj + 17, 0:avn],
                    vaug[:, p_kt, 4 * g2 + lj, :],
                    pt[:, j2 * QC : j2 * QC + avn],
                    start=False,
                    stop=(p_kt == KT - 1),
                    tile_position=(0, 32 * lj),
                ).ins

        at_tiles = {}
        for nn in range(NQ):
            qs = nn * QC
            at = acc.tile([P, QC], F32, tag="acc", name="at")
            zero_fill(at[:], QC)
            at_tiles[nn] = at
            for kt_i in range(KT):
                ksl = slice(kt_i * P, (kt_i + 1) * P)
                for pr in range(2):
                    sc = flow.tile([P, 2 * QC], F32, tag="flow", name="sc")
                    scn = ABLATE["sc_n"]
                    for j2 in range(2):
                        rg = 32 * (2 * pr + j2)
                        last_pe[0] = nc.tensor.matmul(
                            sc[:, j2 * QC : j2 * QC + scn],
                            kt[g2][rg : rg + D, ksl],
                            qt[g2][rg : rg + D, qs : qs + scn],
                            start=True,
                            stop=True,
                            tile_position=(rg, 0),
                        ).ins
                    while len(pending) >= PIPE_DEPTH:
                        flush_one()
                    pt = ptp.tile([P, 2 * QC], BF16, tag="pt", name="pt")
                    en = ABLATE["exp_n"]
                    idx = ((g2 * NQ + nn) * KT + kt_i) * 2 + pr
                    if on_act[idx]:
                        nc.scalar.activation(pt[:, 0:en], sc[:, 0:en], EXPF)
                    else:
                        nc.vector.tensor_scalar(
                            pt[:, 0:en].bitcast(I16), sc[:, 0:en], SCH_A, SCH_B,
                            mybir.AluOpType.mult, mybir.AluOpType.add,
                        )
                    pending.append((pt, at, kt_i, pr))
            if nn > 0:
                epilogue(g2, nn - 1, at_tiles[nn - 1])
        while pending:
            flush_one()
        epilogue(g2, NQ - 1, at_tiles[NQ - 1])

    def epilogue(g2, nn, at):
        # normalize: broadcast the per-head sums row, fast reciprocal,
        # in-place multiply.  Garbage rows stay finite and are killed
        # by the zero rows of sel / wp.
        ot = otp.tile([P, QC], F32R, tag="ot", name=f"ot{g2}{nn}")
        nc.vector.tensor_copy(ot[:], at[:])
        bc = flow.tile([P, QC], F32, tag="flow", name="bc")
        nc.tensor.matmul(bc[:], sel_sb[:], ot[:], start=True, stop=True)
        rec = work.tile([P, QC], F32, tag="rec", name="rec")
        nc.vector.reciprocal_approx_fast(rec[:], bc[:])
        nc.vector.tensor_mul(ot[:], ot[:], rec[:])
        ot_tiles[(g2, nn)] = ot

    def projection():
        out_r = out_d[:].rearrange("(t p) c -> p t c", p=P)
        for nn in range(NQ):
            pp = flow.tile([P, 4 * C], F32, tag="flow", name="pp")
            zero_fill(pp[:, 0:QC], QC)
            zero_fill(pp[:, QC : 2 * QC], QC)
            for ss in range(QC // P):
                for g2 in range(2):
                    nc.tensor.matmul(
                        pp[:, ss * C : (ss + 1) * C],
                        ot_tiles[(g2, nn)][:, ss * P : (ss + 1) * P],
                        wp_sb[:, g2, :],
                        start=False,
                        stop=(g2 == 1),
                    )
            po = work.tile([P, 4 * C], F32, tag="po", name="po")
            nc.vector.tensor_copy(po[:], pp[:])
            for ss in range(QC // P):
                nc.sync.dma_start(
                    out_r[:, nn * (QC // P) + ss, :], po[:, ss * C : (ss + 1) * C]
                )

    prologue_qk(0)
    prologue_v()
    attention(0)
    prologue_qk(1)
    attention(1)
    projection()




def _dedup_ldweights(nc):
    """Remove InstLdweights whose target PE rows already hold the same
    weights.  Identity is content-based: a "row-identity" load (PE row r
    <- SBUF partition r of the same memref/columns) marks every 32-row
    group it covers, so a 64-row covering load later satisfies the
    16-row per-matmul loads inside it.  Only loads whose groups are all
    still resident in the FINAL (post-schedule) order are dropped, so
    scheduler interleaving can reduce the benefit but never correctness.
    Waits/updates of removed loads move onto the following instruction;
    bacc's generate_event_semaphores legalizes multi-wait results."""
    for fn in nc.m.functions:
        for blk in fn.blocks:
            insts = blk.instructions
            resident = {}  # 32-row group -> content id
            keep = []
            for inst in insts:
                tn = type(inst).__name__
                if tn == "InstLdweights":
                    ap = inst.ins[0]
                    tp = inst.tile_position
                    ts = inst.tile_size
                    apl = list(ap.ap)
                    ok = (
                        tp is not None and ts is not None
                        and len(apl) >= 1 and apl[0][0] > 0
                        and inst.perf_mode is None
                        and not inst.is_transpose
                    )
                    if ok:
                        stride0 = apl[0][0]
                        nrows = apl[0][1]
                        pbase = ap.offset // stride0
                        col_off = ap.offset % stride0
                        row_identity = (pbase == tp[0]) and tp[1] == 0
                        content = (
                            str(ap.memref), str(apl[1:]), col_off,
                            str(ap.dtype),
                        )
                        r0 = tp[0] // 32
                        r1 = (tp[0] + nrows + 31) // 32
                        groups = list(range(r0, r1))
                    else:
                        row_identity = False
                        groups = list(range(4))
                        content = None
                    if row_identity and all(
                        resident.get(g) == content for g in groups
                    ):
                        si = inst.sync_info
                        if si is not None and (si.on_wait or si.on_update):
                            keep.append(("MOVE", si))
                        continue
                    for g in groups:
                        resident[g] = content if row_identity else None
                    keep.append(("KEEP", inst))
                else:
                    if tn in ("InstMatmult", "InstMatmultMx"):
                        if getattr(inst, "is_transpose", None):
                            resident.clear()
                    elif tn not in (
                        "InstActivation", "InstTensorCopy", "InstTensorScalarPtr",
                        "InstTensorTensor", "InstMemset", "InstDMACopy",
                        "InstTensorReduce", "InstEventSemaphore", "InstNop",
                        "InstReciprocal", "InstCustomDveAnt", "InstDrain",
                        "InstRegisterMove",
                    ):
                        resident.clear()
                    keep.append(("KEEP", inst))
            new_list = []
            pending_sync = []
            for kind, obj in keep:
                if kind == "MOVE":
                    pending_sync.append(obj)
                    continue
                inst = obj
                if pending_sync:
                    si = inst.sync_info
                    waits = list(si.on_wait) if si is not None else []
                    updates = list(si.on_update) if si is not None else []
                    for ps in pending_sync:
                        waits.extend(ps.on_wait)
                        updates.extend(ps.on_update)
                    inst.sync_info = mybir.SyncInfo(on_wait=waits, on_update=updates)
                    pending_sync = []
                new_list.append(inst)
            if len(new_list) != len(insts):
                blk.instructions[:] = new_list


def _get_nc(reps=1):
    key = (N_ACT, reps)
    if key not in _NC_CACHE:
        _NC_CACHE[key] = build(N_ACT, reps=reps)
    return _NC_CACHE[key]


def make_core_inputs(core, x, w_qkv, b_qkv, w_proj):
    """Host-side sharding: slice/spread weights for one core."""
    b, g = core // 2, core % 2
    wq_s = np.zeros((2, C, P), np.float32)
    wk_s = np.zeros((2, C, P), np.float32)
    bq_s = np.zeros((2, P), np.float32)
    wv_s = np.zeros((C, P), np.float32)
    wp_s = np.zeros((2, P, C), np.float32)
    for g2 in range(2):
        for j in range(4):
            h = 8 * g + 4 * g2 + j
            sp = slice(32 * j, 32 * j + D)
            wq_s[g2, :, sp] = w_qkv[:, 0 * C + h * D : 0 * C + (h + 1) * D]
            wk_s[g2, :, sp] = w_qkv[:, 1 * C + h * D : 1 * C + (h + 1) * D]
            bq_s[g2, sp] = b_qkv[0 * C + h * D : 0 * C + (h + 1) * D]
            wp_s[g2, sp, :] = w_proj[h * D : (h + 1) * D, :]
    for lh in range(8):
        h = 8 * g + lh
        wv_s[:, 16 * lh : 16 * lh + 16] = w_qkv[:, 2 * C + h * D : 2 * C + (h + 1) * D]
    sel = np.zeros((P, P), np.float32)
    for j in range(4):
        sel[32 * j + 16, 32 * j : 32 * j + 32] = 1.0
    # x pre-transposed to channel-major: xt[p, cc, n] = x[b, n, cc*128+p]
    xt = np.ascontiguousarray(
        x[b].T.reshape(CC, P, N_FULL).transpose(1, 0, 2), dtype=np.float32
    )
    return {
        "xt": xt,
        "wq": wq_s, "wk": wk_s, "wv": wv_s,
        "bq": bq_s, "wp": wp_s, "sel": sel,
    }


def kernel(x, w_qkv, b_qkv, w_proj, b_proj):
    global LAST_RESULT
    from concourse.bass_utils import run_bass_kernel_spmd

    x = np.asarray(x, dtype=np.float32)
    w_qkv = np.asarray(w_qkv, dtype=np.float32)
    b_qkv = np.asarray(b_qkv, dtype=np.float32)
    w_proj = np.asarray(w_proj, dtype=np.float32)
    b_proj = np.asarray(b_proj, dtype=np.float32)

    nc = _get_nc(reps=TIMING_REPS)
    in_maps = [
        make_core_inputs(core, x, w_qkv, b_qkv, w_proj) for core in range(NCORES)
    ]
    res = run_bass_kernel_spmd(nc, in_maps, list(range(NCORES)))
    LAST_RESULT = res
    out = np.zeros((B, N_FULL, C), np.float32)
    for core in range(NCORES):
        out[core // 2] += res.results[core]["out"]
    # v-bias folds into the output bias: out += (b_v @ w_proj + b_proj)
    out += (b_qkv[2 * C : 3 * C] @ w_proj + b_proj)[None, None, :]
    return out


# revision 31
# speedup vs baseline: 1.0294x; 1.0294x over previous
"""Trainium2 Bass kernel for multi-head attention (B=4, N=2048, C=256, H=16).

Sharding: 8 cores, core = 2*b + g handles batch b and heads 8g..8g+7 (two
groups g2 of 4 heads).  Each core computes its heads' attention plus a
partial output projection; the host sums the two partials per batch and
adds b_proj (plus the folded v-bias term, see below).

Bottleneck analysis: softmax exp over 8 heads x 2048^2 = 33.5M elements
per core is elementwise-engine bound (1 elem/cycle/lane reading fp32 from
PSUM).  The kernel therefore SPLITS the exp work between ScalarE (true
exp activation) and VectorE (Schraudolph bit-trick exp: bf16 bits =
int16(round(A*s + B)), A = 128*log2(e), B = 128*127).  The bit-trick's
~4% per-element error largely cancels in the softmax normalization
(denominators use the same approximate values); measured end-to-end
rel_fro ~1.1e-2 at a ~39% VectorE share (gate is 2e-2).

Per-core layout (channels on partitions, "transposed"):
  xT  [128, cc, n]  host-pretransposed, DMA'd outside the timed loop
  qT/kT fp16 spread: head lj of a group occupies partitions
      32lj..32lj+16, = W^T @ xT (fp32r self-loading matmuls, copied out
      as fp16);  q gets +bq, k bias is DROPPED (exactly cancels in
      softmax), v bias is folded into the host-side output bias
      (sum(p)=Z normalization makes it additive).
  v natural [tok, vd] via matmul(lhsT=xT-chunk, rhs=Wv-chunk) -> vaug
      bf16 [keys, kt, 8, 17] with ones in column 16 (row-sum trick).
  scores S^T[key, q] per (g2, nn, kt, pr): 2 row-group fp16 matmuls
      (K=16, tile_position=(32lj, 0)) into one [128, 1024] PSUM tile.
  exp -> bf16 P tile: ScalarE activation OR VectorE tensor_scalar into
      an int16 bitcast of the bf16 tile (engine chosen per tile by a
      Bresenham split with N_ACT/256 on ScalarE).
  attnv: col-group bf16 matmuls (tile_position=(0, 32lj)) accumulate
      all 4 heads into ONE PSUM bank per (g2, nn); a K=1 zero-row
      "zero_fill" matmul clears/claims the bank first (start=True wipes
      a whole bank, and the Tile scheduler may reorder disjoint-region
      writers, so per-chain start flags are unsafe); the chains then
      accumulate with start=False, which is reorder-safe.
  Emission is software-pipelined (PIPE_DEPTH): scores for tile t+2 are
      emitted before attnv for tile t so the static per-engine schedule
      keeps PE busy while ScalarE/VectorE exponentiate.
  normalize: sums row broadcast via sel matmul, reciprocal_approx_fast,
      in-place multiply; projection fp32r, partial out DMA'd per token
      tile.  A post-schedule pass (_dedup_ldweights) drops LDWEIGHTS
      that reload weights already resident in the same PE row-group.
"""

import numpy as np

import concourse.bass as bass
import concourse.mybir as mybir
import concourse.tile as tile
from concourse import bacc

F32 = mybir.dt.float32
F32R = mybir.dt.float32r
FP16 = mybir.dt.float16
BF16 = mybir.dt.bfloat16
I16 = mybir.dt.int16
EXPF = mybir.ActivationFunctionType.Exp

P = 128
B, N_FULL, C, H, D = 4, 2048, 256, 16, 16
CC = C // P          # 2 channel chunks
KT = N_FULL // P     # 16 key tiles
QC = 512             # q-chunk
NQ = N_FULL // QC    # 4 q-chunks
NCORES = 8

LOG2E = 1.4426950408889634
SCH_A = float(np.float32(128.0 * LOG2E))
SCH_B = float(np.float32(128.0 * 127.0))

# Number of the 256 exp tiles handled by ScalarE (rest go to VectorE via
# the Schraudolph bit-trick).  Balances ScalarE vs VectorE busy time.
N_ACT = 156

# Ablation knobs (timing experiments only; break numerics when < full size):
# free-dim used by each component's instructions.
ABLATE = {"sc_n": QC, "exp_n": 2 * QC, "av_n": QC}

# Scores-ahead-of-attnv distance in the emitted instruction stream.
PIPE_DEPTH = 2

# q/k + scores matmul dtype: FP16 (discrete LDWEIGHTS path) or F32R
# (self-loading weight path, no separate load instruction).
SC_DT = FP16

_NC_CACHE: dict = {}
LAST_RESULT = None  # BassKernelResults of the most recent run (for test.py)
TIMING_REPS = 1  # >1 repeats the compute on-device (timing); output unchanged


def _act_assignment(n_act=N_ACT):
    """Bresenham-spread a boolean per exp-tile index: True -> ScalarE."""
    flags = []
    for idx in range(256):
        flags.append(((idx + 1) * n_act) // 256 != (idx * n_act) // 256)
    return flags


def build(n_act=N_ACT, reps=1):
    on_act = _act_assignment(n_act)

    nc = bacc.Bacc()
    xt_d = nc.dram_tensor("xt", [P, CC, N_FULL], F32R, kind="ExternalInput")
    wq_d = nc.dram_tensor("wq", [2, C, P], F32R, kind="ExternalInput")
    wk_d = nc.dram_tensor("wk", [2, C, P], F32R, kind="ExternalInput")
    wv_d = nc.dram_tensor("wv", [C, P], F32R, kind="ExternalInput")
    bq_d = nc.dram_tensor("bq", [2, P], F32, kind="ExternalInput")
    wp_d = nc.dram_tensor("wp", [2, P, C], F32R, kind="ExternalInput")
    sel_d = nc.dram_tensor("sel", [P, P], F32R, kind="ExternalInput")
    out_d = nc.dram_tensor("out", [N_FULL, C], F32, kind="ExternalOutput")

    with tile.TileContext(nc) as tc:
        with (
            tc.tile_pool(name="const", bufs=1) as const,
            tc.tile_pool(name="otp", bufs=8) as otp,
            tc.tile_pool(name="work", bufs=4) as work,
            tc.tile_pool(name="ptp", bufs=6) as ptp,
            tc.tile_pool(name="flow", bufs=3, space="PSUM") as flow,
            tc.tile_pool(name="acc", bufs=2, space="PSUM") as acc,
        ):
            # ---------------- loads (outside the timed loop) ----------------
            def staged_load(name, shape, dt, src_ap):
                sb = const.tile(shape, dt, name=f"{name}_sb")
                nc.sync.dma_start(sb[:], src_ap)
                return sb

            wq_sb = staged_load(
                "wq", [P, 2, CC, P], F32R,
                wq_d[:].rearrange("g (cc p) f -> p g cc f", p=P),
            )
            wk_sb = staged_load(
                "wk", [P, 2, CC, P], F32R,
                wk_d[:].rearrange("g (cc p) f -> p g cc f", p=P),
            )
            wv_sb = staged_load(
                "wv", [P, CC, P], F32R, wv_d[:].rearrange("(cc p) f -> p cc f", p=P)
            )
            bq_sb = staged_load("bq", [P, 2], F32, bq_d[:].rearrange("g p -> p g"))
            wp_sb = staged_load("wp", [P, 2, C], F32R, wp_d[:].rearrange("g p c -> p g c"))
            sel_sb = staged_load("sel", [P, P], F32R, sel_d[:])
            # zero row for the PSUM-clearing dummy matmuls (see _build_body)
            zero_sb = const.tile([P, QC], BF16, name="zero_sb")
            nc.vector.memset(zero_sb[:], 0.0)
            # x, pre-transposed on host; chunked DMAs to engage parallel queues
            xt_sb = const.tile([P, CC, N_FULL], F32R, name="xt_sb")
            for cc in range(CC):
                for half in range(2):
                    sl = slice(half * (N_FULL // 2), (half + 1) * (N_FULL // 2))
                    nc.sync.dma_start(xt_sb[:, cc, sl], xt_d[:, cc, sl])

            from contextlib import nullcontext

            loop_ctx = tc.For_i(0, reps, 1) if reps > 1 else nullcontext()
            with loop_ctx:
                _build_body(
                    nc, tc, const, otp, work, ptp, flow, acc, on_act,
                    xt_sb, wq_sb, wk_sb, wv_sb, wp_sb, sel_sb, bq_sb, zero_sb,
                    out_d,
                )
    _dedup_ldweights(nc)
    nc.finalize()
    return nc


def _build_body(
    nc, tc, const, otp, work, ptp, flow, acc, on_act,
    xt_sb, wq_sb, wk_sb, wv_sb, wp_sb, sel_sb, bq_sb, zero_sb, out_d,
):
    def zero_fill(psum_ap, ncols):
        """K=1 matmul of a zero row: clears the bank's has_written bits,
        writes zeros with the bits set over the whole region, and (by
        overlapping every later accumulating matmul) forces WAW ordering.
        Accumulation chains into disjoint regions of a shared bank then use
        start=False throughout, which is reorder-safe (pure adds)."""
        nc.tensor.matmul(
            psum_ap, zero_sb[0:1, 0:psum_ap.partition_size()],
            zero_sb[0:1, 0:ncols], start=True, stop=True,
        )
    qt = [const.tile([P, N_FULL], SC_DT, name=f"qt{g}") for g in range(2)]
    kt = [const.tile([P, N_FULL], SC_DT, name=f"kt{g}") for g in range(2)]
    vaug = const.tile([P, KT, 8, 17], BF16, name="vaug")
    nc.vector.memset(vaug[:, :, :, 16], 1.0)

    def prologue_qk(g2):
        for c in range(NQ):
            sl = slice(c * QC, (c + 1) * QC)
            for w_sb, dst, bias in ((wq_sb, qt[g2], True), (wk_sb, kt[g2], False)):
                ps = flow.tile([P, QC], F32, tag="flow", name="ps")
                for cc in range(CC):
                    nc.tensor.matmul(
                        ps[:],
                        w_sb[:, g2, cc, :],
                        xt_sb[:, cc, sl],
                        start=(cc == 0),
                        stop=(cc == CC - 1),
                    )
                nc.vector.tensor_copy(dst[:, sl], ps[:])
                if bias:
                    nc.vector.tensor_scalar_add(
                        dst[:, sl], dst[:, sl], bq_sb[:, g2 : g2 + 1]
                    )

    def prologue_v():
        for t in range(KT):
            ps = flow.tile([P, P], F32, tag="flow", name="psv")
            for cc in range(CC):
                nc.tensor.matmul(
                    ps[:],
                    xt_sb[:, cc, t * P : (t + 1) * P],
                    wv_sb[:, cc, :],
                    start=(cc == 0),
                    stop=(cc == CC - 1),
                )
            nc.vector.tensor_copy(
                vaug[:, t, :, 0:16], ps[:].rearrange("p (h d) -> p h d", d=16)
            )

    ot_tiles = {}

    def attention(g2):
        # Software-pipelined emission: scores matmuls run PIPE_DEPTH tiles
        # ahead of the attnv matmuls in the static per-engine schedule, so
        # PE computes upcoming scores while ScalarE/VectorE exponentiate
        # and the exp wait is already satisfied at each attnv pair.
        from concourse.tile_rust import add_dep_helper

        pending = []  # [(pt, at, kt_i, pr), ...]
        last_pe = [None]

        def flush_one():
            if not pending:
                return
            pt, p_at, p_kt, p_pr = pending.pop(0)
            avn = ABLATE["av_n"]
            for j2 in range(2):
                lj = 2 * p_pr + j2
                last_pe[0] = nc.tensor.matmul(
                    p_at[32 * lj : 32 * lj + 17, 0:avn],
                    vaug[:, p_kt, 4 * g2 + lj, :],
                    pt[:, j2 * QC : j2 * QC + avn],
                    start=False,
                    stop=(p_kt == KT - 1),
                    tile_position=(0, 32 * lj),
                ).ins

        at_tiles = {}
        for nn in range(NQ):
            qs = nn * QC
            at = acc.tile([P, QC], F32, tag="acc", name="at")
            zero_fill(at[:], QC)
            at_tiles[nn] = at
            for kt_i in range(KT):
                ksl = slice(kt_i * P, (kt_i + 1) * P)
                for pr in range(2):
                    sc = flow.tile([P, 2 * QC], F32, tag="flow", name="sc")
                    scn = ABLATE["sc_n"]
                    for j2 in range(2):
                        rg = 32 * (2 * pr + j2)
                        last_pe[0] = nc.tensor.matmul(
                            sc[:, j2 * QC : j2 * QC + scn],
                            kt[g2][rg : rg + D, ksl],
                            qt[g2][rg : rg + D, qs : qs + scn],
                            start=True,
                            stop=True,
                            tile_position=(rg, 0),
                        ).ins
                    while len(pending) >= PIPE_DEPTH:
                        flush_one()
                    pt = ptp.tile([P, 2 * QC], BF16, tag="pt", name="pt")
                    en = ABLATE["exp_n"]
                    idx = ((g2 * NQ + nn) * KT + kt_i) * 2 + pr
                    if on_act[idx]:
                        nc.scalar.activation(pt[:, 0:en], sc[:, 0:en], EXPF)
                    else:
                        nc.vector.tensor_scalar(
                            pt[:, 0:en].bitcast(I16), sc[:, 0:en], SCH_A, SCH_B,
                            mybir.AluOpType.mult, mybir.AluOpType.add,
                        )
                    pending.append((pt, at, kt_i, pr))
            if nn > 0:
                epilogue(g2, nn - 1, at_tiles[nn - 1])
        while pending:
            flush_one()
        epilogue(g2, NQ - 1, at_tiles[NQ - 1])

    def epilogue(g2, nn, at):
        # normalize: broadcast the per-head sums row, fast reciprocal,
        # in-place multiply.  Garbage rows stay finite and are killed
        # by the zero rows of sel / wp.
        ot = otp.tile([P, QC], F32R, tag="ot", name=f"ot{g2}{nn}")
        nc.vector.tensor_copy(ot[:], at[:])
        bc = flow.tile([P, QC], F32, tag="flow", name="bc")
        nc.tensor.matmul(bc[:], sel_sb[:], ot[:], start=True, stop=True)
        rec = work.tile([P, QC], F32, tag="rec", name="rec")
        nc.vector.reciprocal_approx_fast(rec[:], bc[:])
        nc.vector.tensor_mul(ot[:], ot[:], rec[:])
        ot_tiles[(g2, nn)] = ot

    def projection():
        out_r = out_d[:].rearrange("(t p) c -> p t c", p=P)
        for nn in range(NQ):
            pp = flow.tile([P, 4 * C], F32, tag="flow", name="pp")
            zero_fill(pp[:, 0:QC], QC)
            zero_fill(pp[:, QC : 2 * QC], QC)
            for ss in range(QC // P):
                for g2 in range(2):
                    nc.tensor.matmul(
                        pp[:, ss * C : (ss + 1) * C],
                        ot_tiles[(g2, nn)][:, ss * P : (ss + 1) * P],
                        wp_sb[:, g2, :],
                        start=False,
                        stop=(g2 == 1),
                    )
            po = work.tile([P, 4 * C], F32, tag="po", name="po")
            nc.vector.tensor_copy(po[:], pp[:])
            for ss in range(QC // P):
                nc.sync.dma_start(
                    out_r[:, nn * (QC // P) + ss, :], po[:, ss * C : (ss + 1) * C]
                )

    prologue_qk(0)
    prologue_v()
    attention(0)
    prologue_qk(1)
    attention(1)
    projection()




def _dedup_ldweights(nc):
    """Remove InstLdweights whose target PE rows already hold the same
    weights.  Identity is content-based: a "row-identity" load (PE row r
    <- SBUF partition r of the same memref/columns) marks every 32-row
    group it covers, so a 64-row covering load later satisfies the
    16-row per-matmul loads inside it.  Only loads whose groups are all
    still resident in the FINAL (post-schedule) order are dropped, so
    scheduler interleaving can reduce the benefit but never correctness.
    Waits/updates of removed loads move onto the following instruction;
    bacc's generate_event_semaphores legalizes multi-wait results."""
    for fn in nc.m.functions:
        for blk in fn.blocks:
            insts = blk.instructions
            resident = {}  # 32-row group -> content id
            keep = []
            for inst in insts:
                tn = type(inst).__name__
                if tn == "InstLdweights":
                    ap = inst.ins[0]
                    tp = inst.tile_position
                    ts = inst.tile_size
                    apl = list(ap.ap)
                    ok = (
                        tp is not None and ts is not None
                        and len(apl) >= 1 and apl[0][0] > 0
                        and inst.perf_mode is None
                        and not inst.is_transpose
                    )
                    if ok:
                        stride0 = apl[0][0]
                        nrows = apl[0][1]
                        pbase = ap.offset // stride0
                        col_off = ap.offset % stride0
                        row_identity = (pbase == tp[0]) and tp[1] == 0
                        content = (
                            str(ap.memref), str(apl[1:]), col_off,
                            str(ap.dtype),
                        )
                        r0 = tp[0] // 32
                        r1 = (tp[0] + nrows + 31) // 32
                        groups = list(range(r0, r1))
                    else:
                        row_identity = False
                        groups = list(range(4))
                        content = None
                    if row_identity and all(
                        resident.get(g) == content for g in groups
                    ):
                        si = inst.sync_info
                        if si is not None and (si.on_wait or si.on_update):
                            keep.append(("MOVE", si))
                        continue
                    for g in groups:
                        resident[g] = content if row_identity else None
                    keep.append(("KEEP", inst))
                else:
                    if tn in ("InstMatmult", "InstMatmultMx"):
                        if getattr(inst, "is_transpose", None):
                            resident.clear()
                    elif tn not in (
                        "InstActivation", "InstTensorCopy", "InstTensorScalarPtr",
                        "InstTensorTensor", "InstMemset", "InstDMACopy",
                        "InstTensorReduce", "InstEventSemaphore", "InstNop",
                        "InstReciprocal", "InstCustomDveAnt", "InstDrain",
                        "InstRegisterMove",
                    ):
                        resident.clear()
                    keep.append(("KEEP", inst))
            new_list = []
            pending_sync = []
            for kind, obj in keep:
                if kind == "MOVE":
                    pending_sync.append(obj)
                    continue
                inst = obj
                if pending_sync:
                    si = inst.sync_info
                    waits = list(si.on_wait) if si is not None else []
                    updates = list(si.on_update) if si is not None else []
                    for ps in pending_sync:
                        waits.extend(ps.on_wait)
                        updates.extend(ps.on_update)
                    inst.sync_info = mybir.SyncInfo(on_wait=waits, on_update=updates)
                    pending_sync = []
                new_list.append(inst)
            if len(new_list) != len(insts):
                blk.instructions[:] = new_list


def _get_nc(reps=1):
    key = (N_ACT, reps)
    if key not in _NC_CACHE:
        _NC_CACHE[key] = build(N_ACT, reps=reps)
    return _NC_CACHE[key]


def make_core_inputs(core, x, w_qkv, b_qkv, w_proj):
    """Host-side sharding: slice/spread weights for one core."""
    b, g = core // 2, core % 2
    wq_s = np.zeros((2, C, P), np.float32)
    wk_s = np.zeros((2, C, P), np.float32)
    bq_s = np.zeros((2, P), np.float32)
    wv_s = np.zeros((C, P), np.float32)
    wp_s = np.zeros((2, P, C), np.float32)
    for g2 in range(2):
        for j in range(4):
            h = 8 * g + 4 * g2 + j
            sp = slice(32 * j, 32 * j + D)
            wq_s[g2, :, sp] = w_qkv[:, 0 * C + h * D : 0 * C + (h + 1) * D]
            wk_s[g2, :, sp] = w_qkv[:, 1 * C + h * D : 1 * C + (h + 1) * D]
            bq_s[g2, sp] = b_qkv[0 * C + h * D : 0 * C + (h + 1) * D]
            wp_s[g2, sp, :] = w_proj[h * D : (h + 1) * D, :]
    for lh in range(8):
        h = 8 * g + lh
        wv_s[:, 16 * lh : 16 * lh + 16] = w_qkv[:, 2 * C + h * D : 2 * C + (h + 1) * D]
    sel = np.zeros((P, P), np.float32)
    for j in range(4):
        sel[32 * j + 16, 32 * j : 32 * j + 32] = 1.0
    # x pre-transposed to channel-major: xt[p, cc, n] = x[b, n, cc*128+p]
    xt = np.ascontiguousarray(
        x[b].T.reshape(CC, P, N_FULL).transpose(1, 0, 2), dtype=np.float32
    )
    return {
        "xt": xt,
        "wq": wq_s, "wk": wk_s, "wv": wv_s,
        "bq": bq_s, "wp": wp_s, "sel": sel,
    }


def kernel(x, w_qkv, b_qkv, w_proj, b_proj):
    global LAST_RESULT
    from concourse.bass_utils import run_bass_kernel_spmd

    x = np.asarray(x, dtype=np.float32)
    w_qkv = np.asarray(w_qkv, dtype=np.float32)
    b_qkv = np.asarray(b_qkv, dtype=np.float32)
    w_proj = np.asarray(w_proj, dtype=np.float32)
    b_proj = np.asarray(b_proj, dtype=np.float32)

    nc = _get_nc(reps=TIMING_REPS)
    in_maps = [
        make_core_inputs(core, x, w_qkv, b_qkv, w_proj) for core in range(NCORES)
    ]
    res = run_bass_kernel_spmd(nc, in_maps, list(range(NCORES)))
    LAST_RESULT = res
    out = np.zeros((B, N_FULL, C), np.float32)
    for core in range(NCORES):
        out[core // 2] += res.results[core]["out"]
    # v-bias folds into the output bias: out += (b_v @ w_proj + b_proj)
    out += (b_qkv[2 * C : 3 * C] @ w_proj + b_proj)[None, None, :]
    return out


# revision 34
# speedup vs baseline: 1.2546x; 1.2188x over previous
"""Trainium2 Bass kernel for multi-head attention (B=4, N=2048, C=256, H=16).

Sharding: 8 cores, core = 2*b + g handles batch b and heads 8g..8g+7 (two
groups g2 of 4 heads).  Each core computes its heads' attention plus a
partial output projection; the host sums the two partials per batch and
adds b_proj (plus the folded v-bias term, see below).

Bottleneck analysis: softmax exp over 8 heads x 2048^2 = 33.5M elements
per core is elementwise-engine bound (1 elem/cycle/lane reading fp32 from
PSUM).  The kernel therefore SPLITS the exp work between ScalarE (true
exp activation) and VectorE (Schraudolph bit-trick exp: bf16 bits =
int16(round(A*s + B)), A = 128*log2(e), B = 128*127).  The bit-trick's
~4% per-element error largely cancels in the softmax normalization
(denominators use the same approximate values); measured end-to-end
rel_fro ~1.1e-2 at a ~39% VectorE share (gate is 2e-2).

Per-core layout (channels on partitions, "transposed"):
  xT  [128, cc, n]  host-pretransposed, DMA'd outside the timed loop
  qT/kT fp16 spread: head lj of a group occupies partitions
      32lj..32lj+16, = W^T @ xT (fp32r self-loading matmuls, copied out
      as fp16);  q gets +bq, k bias is DROPPED (exactly cancels in
      softmax), v bias is folded into the host-side output bias
      (sum(p)=Z normalization makes it additive).
  v natural [tok, vd] via matmul(lhsT=xT-chunk, rhs=Wv-chunk) -> vaug
      bf16 [keys, kt, 8, 17] with ones in column 16 (row-sum trick).
  scores S^T[key, q] per (g2, nn, kt, pr): 2 row-group fp16 matmuls
      (K=16, tile_position=(32lj, 0)) into one [128, 1024] PSUM tile.
  exp -> bf16 P tile: ScalarE activation OR VectorE tensor_scalar into
      an int16 bitcast of the bf16 tile (engine chosen per tile by a
      Bresenham split with N_ACT/256 on ScalarE).
  attnv: col-group bf16 matmuls (tile_position=(0, 32lj)) accumulate
      all 4 heads into ONE PSUM bank per (g2, nn); a K=1 zero-row
      "zero_fill" matmul clears/claims the bank first (start=True wipes
      a whole bank, and the Tile scheduler may reorder disjoint-region
      writers, so per-chain start flags are unsafe); the chains then
      accumulate with start=False, which is reorder-safe.
  Emission is software-pipelined (PIPE_DEPTH): scores for tile t+2 are
      emitted before attnv for tile t so the static per-engine schedule
      keeps PE busy while ScalarE/VectorE exponentiate.
  normalize: sums row broadcast via sel matmul, reciprocal_approx_fast,
      in-place multiply; projection fp32r, partial out DMA'd per token
      tile.  A post-schedule pass (_dedup_ldweights) drops LDWEIGHTS
      that reload weights already resident in the same PE row-group.
"""

import numpy as np

import concourse.bass as bass
import concourse.mybir as mybir
import concourse.tile as tile
from concourse import bacc

F32 = mybir.dt.float32
F32R = mybir.dt.float32r
FP16 = mybir.dt.float16
BF16 = mybir.dt.bfloat16
I16 = mybir.dt.int16
EXPF = mybir.ActivationFunctionType.Exp

P = 128
B, N_FULL, C, H, D = 4, 2048, 256, 16, 16
CC = C // P          # 2 channel chunks
KT = N_FULL // P     # 16 key tiles
QC = 512             # q-chunk
NQ = N_FULL // QC    # 4 q-chunks
NCORES = 8

LOG2E = 1.4426950408889634
SCH_A = float(np.float32(128.0 * LOG2E))
SCH_B = float(np.float32(128.0 * 127.0))

# Number of the 256 exp tiles handled by ScalarE (rest go to VectorE via
# the Schraudolph bit-trick).  Balances ScalarE vs VectorE busy time.
N_ACT = 156

# Ablation knobs (timing experiments only; break numerics when < full size):
# free-dim used by each component's instructions.
ABLATE = {"sc_n": QC, "exp_n": 2 * QC, "av_n": QC}

# Scores-ahead-of-attnv distance in the emitted instruction stream.
PIPE_DEPTH = 2

# q/k + scores matmul dtype: FP16 (discrete LDWEIGHTS path) or F32R
# (self-loading weight path, no separate load instruction).
SC_DT = FP16

_NC_CACHE: dict = {}
LAST_RESULT = None  # BassKernelResults of the most recent run (for test.py)
TIMING_REPS = 1  # >1 repeats the compute on-device (timing); output unchanged


def _act_assignment(n_act=N_ACT):
    """Bresenham-spread a boolean per exp-tile index: True -> ScalarE."""
    flags = []
    for idx in range(256):
        flags.append(((idx + 1) * n_act) // 256 != (idx * n_act) // 256)
    return flags


def build(n_act=N_ACT, reps=1):
    on_act = _act_assignment(n_act)

    nc = bacc.Bacc()
    xt_d = nc.dram_tensor("xt", [P, CC, N_FULL], FP16, kind="ExternalInput")
    wq_d = nc.dram_tensor("wq", [2, C, P], FP16, kind="ExternalInput")
    wk_d = nc.dram_tensor("wk", [2, C, P], FP16, kind="ExternalInput")
    wv_d = nc.dram_tensor("wv", [C, P], FP16, kind="ExternalInput")
    bq_d = nc.dram_tensor("bq", [2, P], F32, kind="ExternalInput")
    wp_d = nc.dram_tensor("wp", [2, P, C], BF16, kind="ExternalInput")
    sel_d = nc.dram_tensor("sel", [P, P], BF16, kind="ExternalInput")
    out_d = nc.dram_tensor("out", [N_FULL, C], F32, kind="ExternalOutput")

    with tile.TileContext(nc) as tc:
        with (
            tc.tile_pool(name="const", bufs=1) as const,
            tc.tile_pool(name="otp", bufs=8) as otp,
            tc.tile_pool(name="work", bufs=4) as work,
            tc.tile_pool(name="ptp", bufs=6) as ptp,
            tc.tile_pool(name="flow", bufs=3, space="PSUM") as flow,
            tc.tile_pool(name="acc", bufs=2, space="PSUM") as acc,
        ):
            # ---------------- loads (outside the timed loop) ----------------
            def staged_load(name, shape, dt, src_ap):
                sb = const.tile(shape, dt, name=f"{name}_sb")
                nc.sync.dma_start(sb[:], src_ap)
                return sb

            wq_sb = staged_load(
                "wq", [P, 2, CC, P], FP16,
                wq_d[:].rearrange("g (cc p) f -> p g cc f", p=P),
            )
            wk_sb = staged_load(
                "wk", [P, 2, CC, P], FP16,
                wk_d[:].rearrange("g (cc p) f -> p g cc f", p=P),
            )
            wv_sb = staged_load(
                "wv", [P, CC, P], FP16, wv_d[:].rearrange("(cc p) f -> p cc f", p=P)
            )
            bq_sb = staged_load("bq", [P, 2], F32, bq_d[:].rearrange("g p -> p g"))
            wp_sb = staged_load("wp", [P, 2, C], BF16, wp_d[:].rearrange("g p c -> p g c"))
            sel_sb = staged_load("sel", [P, P], BF16, sel_d[:])
            # zero row for the PSUM-clearing dummy matmuls (see _build_body)
            zero_sb = const.tile([P, QC], BF16, name="zero_sb")
            nc.vector.memset(zero_sb[:], 0.0)
            # x, pre-transposed on host; chunked DMAs to engage parallel queues
            xt_sb = const.tile([P, CC, N_FULL], FP16, name="xt_sb")
            for cc in range(CC):
                for half in range(2):
                    sl = slice(half * (N_FULL // 2), (half + 1) * (N_FULL // 2))
                    nc.sync.dma_start(xt_sb[:, cc, sl], xt_d[:, cc, sl])

            from contextlib import nullcontext

            loop_ctx = tc.For_i(0, reps, 1) if reps > 1 else nullcontext()
            with loop_ctx:
                _build_body(
                    nc, tc, const, otp, work, ptp, flow, acc, on_act,
                    xt_sb, wq_sb, wk_sb, wv_sb, wp_sb, sel_sb, bq_sb, zero_sb,
                    out_d,
                )
    _dedup_ldweights(nc)
    nc.finalize()
    return nc


def _build_body(
    nc, tc, const, otp, work, ptp, flow, acc, on_act,
    xt_sb, wq_sb, wk_sb, wv_sb, wp_sb, sel_sb, bq_sb, zero_sb, out_d,
):
    def zero_fill(psum_ap, ncols):
        """K=1 matmul of a zero row: clears the bank's has_written bits,
        writes zeros with the bits set over the whole region, and (by
        overlapping every later accumulating matmul) forces WAW ordering.
        Accumulation chains into disjoint regions of a shared bank then use
        start=False throughout, which is reorder-safe (pure adds)."""
        nc.tensor.matmul(
            psum_ap, zero_sb[0:1, 0:psum_ap.partition_size()],
            zero_sb[0:1, 0:ncols], start=True, stop=True,
        )
    qt = [const.tile([P, N_FULL], SC_DT, name=f"qt{g}") for g in range(2)]
    kt = [const.tile([P, N_FULL], SC_DT, name=f"kt{g}") for g in range(2)]
    vaug = const.tile([P, KT, 8, 17], BF16, name="vaug")
    nc.vector.memset(vaug[:, :, :, 16], 1.0)

    def prologue_qk(g2):
        for c in range(NQ):
            sl = slice(c * QC, (c + 1) * QC)
            for w_sb, dst, bias in ((wq_sb, qt[g2], True), (wk_sb, kt[g2], False)):
                ps = flow.tile([P, QC], F32, tag="flow", name="ps")
                for cc in range(CC):
                    nc.tensor.matmul(
                        ps[:],
                        w_sb[:, g2, cc, :],
                        xt_sb[:, cc, sl],
                        start=(cc == 0),
                        stop=(cc == CC - 1),
                    )
                nc.vector.tensor_copy(dst[:, sl], ps[:])
                if bias:
                    nc.vector.tensor_scalar_add(
                        dst[:, sl], dst[:, sl], bq_sb[:, g2 : g2 + 1]
                    )

    def prologue_v():
        for t in range(KT):
            ps = flow.tile([P, P], F32, tag="flow", name="psv")
            for cc in range(CC):
                nc.tensor.matmul(
                    ps[:],
                    xt_sb[:, cc, t * P : (t + 1) * P],
                    wv_sb[:, cc, :],
                    start=(cc == 0),
                    stop=(cc == CC - 1),
                )
            nc.vector.tensor_copy(
                vaug[:, t, :, 0:16], ps[:].rearrange("p (h d) -> p h d", d=16)
            )

    ot_tiles = {}

    def attention(g2):
        # Software-pipelined emission: scores matmuls run PIPE_DEPTH tiles
        # ahead of the attnv matmuls in the static per-engine schedule, so
        # PE computes upcoming scores while ScalarE/VectorE exponentiate
        # and the exp wait is already satisfied at each attnv pair.
        from concourse.tile_rust import add_dep_helper

        pending = []  # [(pt, at, kt_i, pr), ...]
        last_pe = [None]

        def flush_one():
            if not pending:
                return
            pt, p_at, p_kt, p_pr = pending.pop(0)
            avn = ABLATE["av_n"]
            for j2 in range(2):
                lj = 2 * p_pr + j2
                last_pe[0] = nc.tensor.matmul(
                    p_at[32 * lj : 32 * lj + 17, 0:avn],
                    vaug[:, p_kt, 4 * g2 + lj, :],
                    pt[:, j2 * QC : j2 * QC + avn],
                    start=False,
                    stop=(p_kt == KT - 1),
                    tile_position=(0, 32 * lj),
                ).ins

        at_tiles = {}
        for nn in range(NQ):
            qs = nn * QC
            at = acc.tile([P, QC], F32, tag="acc", name="at")
            zero_fill(at[:], QC)
            at_tiles[nn] = at
            for kt_i in range(KT):
                ksl = slice(kt_i * P, (kt_i + 1) * P)
                for pr in range(2):
                    sc = flow.tile([P, 2 * QC], F32, tag="flow", name="sc")
                    scn = ABLATE["sc_n"]
                    for j2 in range(2):
                        rg = 32 * (2 * pr + j2)
                        last_pe[0] = nc.tensor.matmul(
                            sc[:, j2 * QC : j2 * QC + scn],
                            kt[g2][rg : rg + D, ksl],
                            qt[g2][rg : rg + D, qs : qs + scn],
                            start=True,
                            stop=True,
                            tile_position=(rg, 0),
                        ).ins
                    while len(pending) >= PIPE_DEPTH:
                        flush_one()
                    pt = ptp.tile([P, 2 * QC], BF16, tag="pt", name="pt")
                    en = ABLATE["exp_n"]
                    idx = ((g2 * NQ + nn) * KT + kt_i) * 2 + pr
                    if on_act[idx]:
                        nc.scalar.activation(pt[:, 0:en], sc[:, 0:en], EXPF)
                    else:
                        nc.vector.tensor_scalar(
                            pt[:, 0:en].bitcast(I16), sc[:, 0:en], SCH_A, SCH_B,
                            mybir.AluOpType.mult, mybir.AluOpType.add,
                        )
                    pending.append((pt, at, kt_i, pr))
            if nn > 0:
                epilogue(g2, nn - 1, at_tiles[nn - 1])
        while pending:
            flush_one()
        epilogue(g2, NQ - 1, at_tiles[NQ - 1])

    def epilogue(g2, nn, at):
        # normalize: broadcast the per-head sums row, fast reciprocal,
        # in-place multiply.  Garbage rows stay finite and are killed
        # by the zero rows of sel / wp.
        ot = otp.tile([P, QC], BF16, tag="ot", name=f"ot{g2}{nn}")
        nc.vector.tensor_copy(ot[:], at[:])
        bc = flow.tile([P, QC], F32, tag="flow", name="bc")
        nc.tensor.matmul(bc[:], sel_sb[:], ot[:], start=True, stop=True)
        rec = work.tile([P, QC], F32, tag="rec", name="rec")
        nc.vector.reciprocal_approx_fast(rec[:], bc[:])
        nc.vector.tensor_mul(ot[:], ot[:], rec[:])
        ot_tiles[(g2, nn)] = ot

    def projection():
        out_r = out_d[:].rearrange("(t p) c -> p t c", p=P)
        for nn in range(NQ):
            pp = flow.tile([P, 4 * C], F32, tag="flow", name="pp")
            zero_fill(pp[:, 0:QC], QC)
            zero_fill(pp[:, QC : 2 * QC], QC)
            for ss in range(QC // P):
                for g2 in range(2):
                    nc.tensor.matmul(
                        pp[:, ss * C : (ss + 1) * C],
                        ot_tiles[(g2, nn)][:, ss * P : (ss + 1) * P],
                        wp_sb[:, g2, :],
                        start=False,
                        stop=(g2 == 1),
                    )
            po = work.tile([P, 4 * C], F32, tag="po", name="po")
            nc.vector.tensor_copy(po[:], pp[:])
            for ss in range(QC // P):
                nc.sync.dma_start(
                    out_r[:, nn * (QC // P) + ss, :], po[:, ss * C : (ss + 1) * C]
                )

    prologue_qk(0)
    prologue_v()
    attention(0)
    prologue_qk(1)
    attention(1)
    projection()




def _dedup_ldweights(nc):
    """Remove InstLdweights whose target PE rows already hold the same
    weights.  Identity is content-based: a "row-identity" load (PE row r
    <- SBUF partition r of the same memref/columns) marks every 32-row
    group it covers, so a 64-row covering load later satisfies the
    16-row per-matmul loads inside it.  Only loads whose groups are all
    still resident in the FINAL (post-schedule) order are dropped, so
    scheduler interleaving can reduce the benefit but never correctness.
    Waits/updates of removed loads move onto the following instruction;
    bacc's generate_event_semaphores legalizes multi-wait results."""
    for fn in nc.m.functions:
        for blk in fn.blocks:
            insts = blk.instructions
            resident = {}  # 32-row group -> content id
            keep = []
            for inst in insts:
                tn = type(inst).__name__
                if tn == "InstLdweights":
                    ap = inst.ins[0]
                    tp = inst.tile_position
                    ts = inst.tile_size
                    apl = list(ap.ap)
                    ok = (
                        tp is not None and ts is not None
                        and len(apl) >= 1 and apl[0][0] > 0
                        and inst.perf_mode is None
                        and not inst.is_transpose
                    )
                    if ok:
                        stride0 = apl[0][0]
                        nrows = apl[0][1]
                        pbase = ap.offset // stride0
                        col_off = ap.offset % stride0
                        row_identity = (pbase == tp[0]) and tp[1] == 0
                        content = (
                            str(ap.memref), str(apl[1:]), col_off,
                            str(ap.dtype),
                        )
                        r0 = tp[0] // 32
                        r1 = (tp[0] + nrows + 31) // 32
                        groups = list(range(r0, r1))
                    else:
                        row_identity = False
                        groups = list(range(4))
                        content = None
                    if row_identity and all(
                        resident.get(g) == content for g in groups
                    ):
                        si = inst.sync_info
                        if si is not None and (si.on_wait or si.on_update):
                            keep.append(("MOVE", si))
                        continue
                    for g in groups:
                        resident[g] = content if row_identity else None
                    keep.append(("KEEP", inst))
                else:
                    if tn in ("InstMatmult", "InstMatmultMx"):
                        if getattr(inst, "is_transpose", None):
                            resident.clear()
                    elif tn not in (
                        "InstActivation", "InstTensorCopy", "InstTensorScalarPtr",
                        "InstTensorTensor", "InstMemset", "InstDMACopy",
                        "InstTensorReduce", "InstEventSemaphore", "InstNop",
                        "InstReciprocal", "InstCustomDveAnt", "InstDrain",
                        "InstRegisterMove",
                    ):
                        resident.clear()
                    keep.append(("KEEP", inst))
            new_list = []
            pending_sync = []
            for kind, obj in keep:
                if kind == "MOVE":
                    pending_sync.append(obj)
                    continue
                inst = obj
                if pending_sync:
                    si = inst.sync_info
                    waits = list(si.on_wait) if si is not None else []
                    updates = list(si.on_update) if si is not None else []
                    for ps in pending_sync:
                        waits.extend(ps.on_wait)
                        updates.extend(ps.on_update)
                    inst.sync_info = mybir.SyncInfo(on_wait=waits, on_update=updates)
                    pending_sync = []
                new_list.append(inst)
            if len(new_list) != len(insts):
                blk.instructions[:] = new_list


def _get_nc(reps=1):
    key = (N_ACT, reps)
    if key not in _NC_CACHE:
        _NC_CACHE[key] = build(N_ACT, reps=reps)
    return _NC_CACHE[key]


def make_core_inputs(core, x, w_qkv, b_qkv, w_proj):
    """Host-side sharding: slice/spread weights for one core."""
    b, g = core // 2, core % 2
    wq_s = np.zeros((2, C, P), np.float32)
    wk_s = np.zeros((2, C, P), np.float32)
    bq_s = np.zeros((2, P), np.float32)
    wv_s = np.zeros((C, P), np.float32)
    wp_s = np.zeros((2, P, C), np.float32)
    for g2 in range(2):
        for j in range(4):
            h = 8 * g + 4 * g2 + j
            sp = slice(32 * j, 32 * j + D)
            wq_s[g2, :, sp] = w_qkv[:, 0 * C + h * D : 0 * C + (h + 1) * D]
            wk_s[g2, :, sp] = w_qkv[:, 1 * C + h * D : 1 * C + (h + 1) * D]
            bq_s[g2, sp] = b_qkv[0 * C + h * D : 0 * C + (h + 1) * D]
            wp_s[g2, sp, :] = w_proj[h * D : (h + 1) * D, :]
    for lh in range(8):
        h = 8 * g + lh
        wv_s[:, 16 * lh : 16 * lh + 16] = w_qkv[:, 2 * C + h * D : 2 * C + (h + 1) * D]
    sel = np.zeros((P, P), np.float32)
    for j in range(4):
        sel[32 * j + 16, 32 * j : 32 * j + 32] = 1.0
    # x pre-transposed to channel-major: xt[p, cc, n] = x[b, n, cc*128+p]
    xt = np.ascontiguousarray(
        x[b].T.reshape(CC, P, N_FULL).transpose(1, 0, 2), dtype=np.float32
    )
    f16 = np.float16
    import ml_dtypes
    def bf16(a):
        return a.astype(ml_dtypes.bfloat16)
    return {
        "xt": xt.astype(f16),
        "wq": wq_s.astype(f16), "wk": wk_s.astype(f16), "wv": wv_s.astype(f16),
        "bq": bq_s, "wp": bf16(wp_s), "sel": bf16(sel),
    }


def kernel(x, w_qkv, b_qkv, w_proj, b_proj):
    global LAST_RESULT
    from concourse.bass_utils import run_bass_kernel_spmd

    x = np.asarray(x, dtype=np.float32)
    w_qkv = np.asarray(w_qkv, dtype=np.float32)
    b_qkv = np.asarray(b_qkv, dtype=np.float32)
    w_proj = np.asarray(w_proj, dtype=np.float32)
    b_proj = np.asarray(b_proj, dtype=np.float32)

    nc = _get_nc(reps=TIMING_REPS)
    in_maps = [
        make_core_inputs(core, x, w_qkv, b_qkv, w_proj) for core in range(NCORES)
    ]
    res = run_bass_kernel_spmd(nc, in_maps, list(range(NCORES)))
    LAST_RESULT = res
    out = np.zeros((B, N_FULL, C), np.float32)
    for core in range(NCORES):
        out[core // 2] += res.results[core]["out"]
    # v-bias folds into the output bias: out += (b_v @ w_proj + b_proj)
    out += (b_qkv[2 * C : 3 * C] @ w_proj + b_proj)[None, None, :]
    return out
